# revision 17
# baseline (speedup 1.0000x reference)
"""Trainium2 Bass kernel for nn_NetworkRNNCell (gnn message passing).

Contract: kernel(**inputs) takes FULL unsharded numpy inputs (as produced by
setup_inputs()) and returns the FULL output tuple (out, v_new, s_new),
matching reference() exactly in shapes/dtypes.

Sharding: the unit axis N (=4096) of the synaptic weight matrices W[S,N,N]
is split column-wise across the 8 NeuronCores (512 output columns per core).
Every core processes all S=20 synapses for its column slice:
  - drive[s, j] = sum_i pre[s,i] * W[s,i,j]   (PE matmuls, i tiled by 128)
  - synapse update, conductance currents, segment-sum onto the P=4 target
    populations (one tiny 0/1 selection-matrix matmul), Euler integration,
    output rates -- all elementwise over the local j slice.
This needs no cross-core collectives: the segment-sum runs over the synapse
axis which stays fully local. The only global reduction is the scalar
stability error; each core emits 4 partial sums which the host combines
during unshard.
"""

import sys
import types

import numpy as np

import concourse.bacc as bacc
import concourse.bass as bass
import concourse.tile as tile
from concourse import mybir
from concourse.bass_utils import run_bass_kernel_spmd

# network constants
DT = 0.1
TAU_M = 10.0
TAU_S = 5.0
P = 4
S = 20
N = 4096
B = 1
IN_FREQ = 0.008

N_CORES = 8
JC = N // N_CORES          # output columns per core (512)
KT = N // 128              # contraction tiles of 128 (32)
CH = 8                     # k-tiles per W DMA chunk (8 -> 2 MB chunks)
W_BUFS = 6                 # W tile buffering depth

DECAY = 1.0 - DT / TAU_S   # 0.98

# "f32r": PE fast fp32 mode (full speed, slightly relaxed precision)
# "f32" : exact fp32 matmul (4x PE cycles, still near the DMA roofline)
W_MODE = "f32r"

# Set by test.py to capture an NTFF profile; LAST_EXEC_NS then holds the
# max-over-cores NEFF execution time of the last run.
TRACE = False
LAST_EXEC_NS = None
LAST_RESULTS = None

_PROGRAM_CACHE = {}


def _install_ntff_hook():
    """Provide antenv.axon_hooks (absent on this image) and register the
    NTFF profile hook exposed by the axon boot shim."""
    if "antenv.axon_hooks" not in sys.modules:
        import antenv

        mod = types.ModuleType("antenv.axon_hooks")
        holder = [None]
        mod.set_axon_ntff_profile_hook = lambda h: holder.__setitem__(0, h)
        mod.get_axon_ntff_profile_hook = lambda: holder[0]
        sys.modules["antenv.axon_hooks"] = mod
        antenv.axon_hooks = mod
    import antenv.axon_hooks as ah

    if ah.get_axon_ntff_profile_hook() is None:
        from trn_agent_boot.trn_boot import _ntff_profile_via_ctypes

        ah.set_axon_ntff_profile_hook(
            _ntff_profile_via_ctypes("/opt/axon/libaxon_pjrt.so")
        )


def _build_program(src_idx: tuple, tgt_idx: tuple, mode: str):
    """Build + bacc-compile the SPMD Bass program (identical on all cores)."""
    f32 = mybir.dt.float32
    wdt = mybir.dt.float32r if mode == "f32r" else f32

    nc = bacc.Bacc("TRN2", target_bir_lowering=False, debug=False,
                   num_devices=N_CORES)

    w_in = nc.dram_tensor("w", [128, S, KT, JC], wdt, kind="ExternalInput").ap()
    vt_in = nc.dram_tensor("vt", [128, P, KT], f32, kind="ExternalInput").ap()
    ph_in = nc.dram_tensor("ph", [128, KT], f32, kind="ExternalInput").ap()
    t_in = nc.dram_tensor("t", [1, 1], f32, kind="ExternalInput").ap()
    s_in = nc.dram_tensor("s", [S, JC], f32, kind="ExternalInput").ap()
    v_in = nc.dram_tensor("v", [P, JC], f32, kind="ExternalInput").ap()
    pv_in = nc.dram_tensor("pv", [S, JC], f32, kind="ExternalInput").ap()
    er_in = nc.dram_tensor("er", [S, 1], f32, kind="ExternalInput").ap()
    ms_in = nc.dram_tensor("ms", [S, P], f32, kind="ExternalInput").ap()

    snew_out = nc.dram_tensor("snew", [S, JC], f32, kind="ExternalOutput").ap()
    vnew_out = nc.dram_tensor("vnew", [P, JC], f32, kind="ExternalOutput").ap()
    rates_out = nc.dram_tensor("rates", [P, JC], f32, kind="ExternalOutput").ap()
    errp_out = nc.dram_tensor("errp", [P, 1], f32, kind="ExternalOutput").ap()

    with tile.TileContext(nc) as tc:
        with (
            tc.tile_pool(name="const", bufs=1) as cpool,
            tc.tile_pool(name="wpool", bufs=W_BUFS) as wpool,
            tc.tile_pool(name="stage", bufs=4) as stage_pool,
            tc.tile_pool(name="dpsum", bufs=4, space="PSUM") as dpsum,
            tc.tile_pool(name="apsum", bufs=2, space="PSUM") as apsum,
        ):
            # ---- small resident tiles -------------------------------------
            rates_sb = cpool.tile([128, P + 1, KT], wdt)   # all_rates, lhsT layout
            vt_sb = cpool.tile([128, P, KT], f32)
            ph_sb = cpool.tile([128, KT], f32)
            t_sb = cpool.tile([128, 1], f32)
            pib_sb = cpool.tile([128, 1], f32)
            pib3_sb = cpool.tile([128, 1], f32)
            sin_sb = cpool.tile([128, KT], f32)
            s1_sb = cpool.tile([128, KT], f32)
            s2_sb = cpool.tile([128, KT], f32)
            # synapse rows split 16/4: the tail group keeps the last
            # synapses' work off the critical path, and separate tiles keep
            # every engine AP starting at partition 0
            SA = 16
            SB = S - SA
            s_a = cpool.tile([SA, JC], f32)
            s_b = cpool.tile([SB, JC], f32)
            v_sb = cpool.tile([P, JC], f32)
            pv_a = cpool.tile([SA, JC], f32)
            pv_b = cpool.tile([SB, JC], f32)
            er_a = cpool.tile([SA, 1], f32)
            er_b = cpool.tile([SB, 1], f32)
            ms_a = cpool.tile([SA, P], f32)
            ms_b = cpool.tile([SB, P], f32)
            snew_a = cpool.tile([SA, JC], f32)
            snew_b = cpool.tile([SB, JC], f32)
            sdec_a = cpool.tile([SA, JC], f32)
            sdec_b = cpool.tile([SB, JC], f32)
            wterm_a = cpool.tile([SA, JC], f32)
            wterm_b = cpool.tile([SB, JC], f32)
            isyn_a = cpool.tile([SA, JC], f32)
            isyn_b = cpool.tile([SB, JC], f32)
            gv_sb = cpool.tile([P, JC], f32)
            dv_sb = cpool.tile([P, JC], f32)
            vnew_sb = cpool.tile([P, JC], f32)
            rates4_sb = cpool.tile([P, JC], f32)
            err_sb = cpool.tile([P, 1], f32)

            # all small input DMAs ride the SWDGE path so the sync HWDGE
            # FIFO carries nothing but the back-to-back W stream
            nc.gpsimd.dma_start(out=vt_sb, in_=vt_in)
            nc.gpsimd.dma_start(out=ph_sb, in_=ph_in)
            nc.gpsimd.dma_start(out=s_a, in_=s_in[0:SA, :])
            nc.gpsimd.dma_start(out=s_b, in_=s_in[SA:S, :])
            nc.gpsimd.dma_start(out=v_sb, in_=v_in)
            nc.gpsimd.dma_start(out=pv_a, in_=pv_in[0:SA, :])
            nc.gpsimd.dma_start(out=pv_b, in_=pv_in[SA:S, :])
            nc.gpsimd.dma_start(out=er_a, in_=er_in[0:SA, :])
            nc.gpsimd.dma_start(out=er_b, in_=er_in[SA:S, :])
            nc.gpsimd.dma_start(out=ms_a, in_=ms_in[0:SA, :])
            nc.gpsimd.dma_start(out=ms_b, in_=ms_in[SA:S, :])
            # broadcast t over the 128 partitions
            nc.gpsimd.dma_start(out=t_sb, in_=t_in.to_broadcast((128, 1)))

            # ---- firing rates of all source populations -------------------
            # rows 0..P-1: sigmoid(v); row P: 0.5*(1+sin(2*pi*f*t + phase))
            nc.scalar.activation(
                out=rates_sb[:, 0:P, :], in_=vt_sb,
                func=mybir.ActivationFunctionType.Sigmoid,
            )
            # ScalarE Sin needs args in [-pi, pi]. a = phase + t_red lies in
            # [0, 4pi) (host pre-reduces the 2*pi*f*t scalar mod 2pi), so
            # subtract 2*pi*k with k = (sign(a-pi) + sign(a-3pi))/2 + 1.
            nc.vector.memset(pib_sb, -float(np.pi))
            nc.vector.memset(pib3_sb, -float(3.0 * np.pi))
            nc.vector.tensor_scalar(
                out=sin_sb, in0=ph_sb, scalar1=t_sb, scalar2=None,
                op0=mybir.AluOpType.add,
            )
            nc.scalar.activation(
                out=s1_sb, in_=sin_sb,
                func=mybir.ActivationFunctionType.Sign, bias=pib_sb,
            )
            nc.scalar.activation(
                out=s2_sb, in_=sin_sb,
                func=mybir.ActivationFunctionType.Sign, bias=pib3_sb,
            )
            nc.vector.tensor_add(s1_sb, s1_sb, s2_sb)
            nc.vector.tensor_scalar(
                out=s1_sb, in0=s1_sb,
                scalar1=-float(np.pi), scalar2=-float(2.0 * np.pi),
                op0=mybir.AluOpType.mult, op1=mybir.AluOpType.add,
            )
            nc.vector.tensor_add(sin_sb, sin_sb, s1_sb)
            nc.scalar.activation(
                out=sin_sb, in_=sin_sb,
                func=mybir.ActivationFunctionType.Sin,
            )
            nc.vector.tensor_scalar(
                out=rates_sb[:, P, :], in0=sin_sb,
                scalar1=0.5, scalar2=0.5,
                op0=mybir.AluOpType.mult, op1=mybir.AluOpType.add,
            )

            # ---- per-synapse drive matmuls + synapse update ---------------
            for s in range(S):
                r = src_idx[s]
                drive_ps = dpsum.tile([1, JC], f32, tag="drive")
                for c0 in range(0, KT, CH):
                    w_tile = wpool.tile([128, CH, JC], wdt, tag="w")
                    nc.sync.dma_start(out=w_tile, in_=w_in[:, s, c0:c0 + CH, :])
                    for k in range(CH):
                        kk = c0 + k
                        nc.tensor.matmul(
                            drive_ps, rates_sb[:, r, kk:kk + 1], w_tile[:, k, :],
                            start=(kk == 0), stop=(kk == KT - 1),
                        )
                # s_new partial: DT * drive. ACT stages PSUM -> SBUF at
                # partition 0 (engines can't start mid-partition), then a
                # tiny SBUF->SBUF DMA scatters it to row s.
                stage_sb = stage_pool.tile([1, JC], f32, tag="stage")
                nc.scalar.mul(stage_sb, drive_ps, DT)
                # SWDGE path: keeps this dependent little DMA out of the
                # HWDGE FIFOs that stream the W chunks
                if s < SA:
                    nc.gpsimd.dma_start(out=snew_a[s:s + 1, :], in_=stage_sb)
                else:
                    nc.gpsimd.dma_start(out=snew_b[s - SA:s - SA + 1, :],
                                        in_=stage_sb)

            # ---- synapse state + currents (two row groups; group a runs
            # while the last synapses are still streaming) -------------------
            nc.vector.tensor_scalar_mul(sdec_a, s_a, DECAY)
            nc.vector.tensor_scalar_mul(sdec_b, s_b, DECAY)
            # wterm = E_rev - post_v
            nc.vector.tensor_scalar(
                out=wterm_a, in0=pv_a, scalar1=-1.0, scalar2=er_a,
                op0=mybir.AluOpType.mult, op1=mybir.AluOpType.add,
            )
            nc.vector.tensor_scalar(
                out=wterm_b, in0=pv_b, scalar1=-1.0, scalar2=er_b,
                op0=mybir.AluOpType.mult, op1=mybir.AluOpType.add,
            )
            itot_ps = apsum.tile([P, JC], f32, tag="acc")
            gtot_ps = apsum.tile([P, JC], f32, tag="acc")

            nc.vector.tensor_add(snew_a, snew_a, sdec_a)
            nc.sync.dma_start(out=snew_out[0:SA, :], in_=snew_a)
            nc.vector.tensor_mul(isyn_a, snew_a, wterm_a)
            nc.tensor.matmul(itot_ps, ms_a, isyn_a, start=True, stop=False)
            nc.tensor.matmul(gtot_ps, ms_a, snew_a, start=True, stop=False)

            nc.vector.tensor_add(snew_b, snew_b, sdec_b)
            nc.sync.dma_start(out=snew_out[SA:S, :], in_=snew_b)
            nc.vector.tensor_mul(isyn_b, snew_b, wterm_b)
            nc.tensor.matmul(itot_ps, ms_b, isyn_b, start=False, stop=True)
            nc.tensor.matmul(gtot_ps, ms_b, snew_b, start=False, stop=True)

            # ---- population integration -----------------------------------
            nc.vector.tensor_mul(gv_sb, gtot_ps, v_sb)
            nc.vector.tensor_sub(dv_sb, itot_ps, gv_sb)
            nc.vector.tensor_sub(dv_sb, dv_sb, v_sb)
            nc.vector.tensor_scalar_mul(dv_sb, dv_sb, 1.0 / TAU_M)
            nc.vector.tensor_reduce(
                out=err_sb, in_=dv_sb, axis=mybir.AxisListType.X,
                op=mybir.AluOpType.add, apply_absolute_value=True,
            )
            nc.vector.tensor_scalar_mul(vnew_sb, dv_sb, DT)
            nc.vector.tensor_add(vnew_sb, vnew_sb, v_sb)
            nc.scalar.activation(
                out=rates4_sb, in_=vnew_sb,
                func=mybir.ActivationFunctionType.Sigmoid,
            )

            nc.sync.dma_start(out=vnew_out, in_=vnew_sb)
            nc.sync.dma_start(out=rates_out, in_=rates4_sb)
            nc.sync.dma_start(out=errp_out, in_=err_sb)

    nc.compile()
    return nc


def _get_program(src_idx, tgt_idx, mode):
    key = (tuple(int(x) for x in src_idx), tuple(int(x) for x in tgt_idx), mode)
    if key not in _PROGRAM_CACHE:
        _PROGRAM_CACHE[key] = _build_program(key[0], key[1], mode)
    return _PROGRAM_CACHE[key]


def _prep_inputs(t, v, s, W, E_rev, phase, src_idx, tgt_idx):
    """Host-side shard/layout prep. No math beyond gathers/transposes."""
    f32 = np.float32
    v2 = np.asarray(v, f32)[:, 0, :]                      # [P, N]
    s2 = np.asarray(s, f32)[:, 0, :]                      # [S, N]
    W = np.asarray(W, f32)
    tgt = np.asarray(tgt_idx)

    # lhsT layouts: element [p, ..., k] = x[..., k*128 + p]
    vt = np.ascontiguousarray(v2.reshape(P, KT, 128).transpose(2, 0, 1))
    ph = np.ascontiguousarray(
        np.asarray(phase, f32).reshape(KT, 128).transpose(1, 0))
    msel = np.zeros((S, P), f32)
    msel[np.arange(S), tgt] = 1.0
    erev = np.asarray(E_rev, f32).reshape(S, 1)
    # t enters as the fp32 angle 2*pi*f*t, pre-reduced mod 2pi (one host
    # scalar op) so the on-device sin range reduction only spans [0, 4pi).
    tr = f32(2.0 * np.pi * IN_FREQ) * np.asarray(t, f32).reshape(-1)[0]
    if tr >= f32(2.0 * np.pi) or tr < 0.0:
        tr = f32(np.float64(tr) % (2.0 * np.pi))
    t_arr = np.asarray(tr, f32).reshape(1, 1)

    # W[s, i, j] with i = kt*128 + p, j = c*JC + jj  ->  per-core [128, S, KT, JC]
    W5 = W.reshape(S, KT, 128, N_CORES, JC)

    in_maps = []
    for c in range(N_CORES):
        wc = np.ascontiguousarray(W5[:, :, :, c, :].transpose(2, 0, 1, 3))
        sl = slice(c * JC, (c + 1) * JC)
        vc = np.ascontiguousarray(v2[:, sl])
        in_maps.append(dict(
            w=wc, vt=vt, ph=ph, t=t_arr,
            s=np.ascontiguousarray(s2[:, sl]),
            v=vc,
            pv=np.ascontiguousarray(vc[tgt]),
            er=erev, ms=msel,
        ))
    return in_maps


def kernel(t, v, s, W, E_rev, phase, src_idx, tgt_idx):
    global LAST_EXEC_NS, LAST_RESULTS
    nc = _get_program(src_idx, tgt_idx, W_MODE)
    in_maps = _prep_inputs(t, v, s, W, E_rev, phase, src_idx, tgt_idx)

    kwargs = {}
    if TRACE:
        _install_ntff_hook()
        kwargs = dict(trace=True, trace_cores=list(range(N_CORES)))
    res = run_bass_kernel_spmd(nc, in_maps, core_ids=list(range(N_CORES)),
                               **kwargs)
    LAST_EXEC_NS = res.exec_time_ns
    LAST_RESULTS = res

    # ---- unshard -----------------------------------------------------------
    f32 = np.float32
    s_new = np.empty((S, B, N), f32)
    v_new = np.empty((P, B, N), f32)
    rates = np.empty((P, N), f32)
    err_acc = 0.0
    for c in range(N_CORES):
        r = res.results[c]
        sl = slice(c * JC, (c + 1) * JC)
        s_new[:, 0, sl] = r["snew"]
        v_new[:, 0, sl] = r["vnew"]
        rates[:, sl] = r["rates"]
        err_acc += float(r["errp"].sum())
    err = np.asarray(err_acc / N, f32)
    out = np.concatenate(
        [rates.reshape(1, P * N), err.reshape(1, 1)], axis=1).astype(f32)
    return out, v_new, s_new


# revision 24
# speedup vs baseline: 1.1846x; 1.1846x over previous
"""Trainium2 Bass kernel for nn_NetworkRNNCell (gnn message passing).

Contract: kernel(**inputs) takes FULL unsharded numpy inputs (as produced by
setup_inputs()) and returns the FULL output tuple (out, v_new, s_new),
matching reference() exactly in shapes/dtypes.

Sharding: the unit axis N (=4096) of the synaptic weight matrices W[S,N,N]
is split column-wise across the 8 NeuronCores (512 output columns per core).
Every core processes all S=20 synapses for its column slice:
  - drive[s, j] = sum_i pre[s,i] * W[s,i,j]   (PE matmuls, i tiled by 128)
  - synapse update, conductance currents, segment-sum onto the P=4 target
    populations (one tiny 0/1 selection-matrix matmul), Euler integration,
    output rates -- all elementwise over the local j slice.
This needs no cross-core collectives: the segment-sum runs over the synapse
axis which stays fully local. The only global reduction is the scalar
stability error; each core emits 4 partial sums which the host combines
during unshard.
"""

import sys
import types

import numpy as np

import concourse.bacc as bacc
import concourse.bass as bass
import concourse.tile as tile
from concourse import mybir
from concourse.bass_utils import run_bass_kernel_spmd

# network constants
DT = 0.1
TAU_M = 10.0
TAU_S = 5.0
P = 4
S = 20
N = 4096
B = 1
IN_FREQ = 0.008

N_CORES = 8
JC = N // N_CORES          # output columns per core (512)
KT = N // 128              # contraction tiles of 128 (32)
CH = 8                     # k-tiles per W DMA chunk (8 -> 2 MB chunks)
W_BUFS = 6                 # W tile buffering depth

DECAY = 1.0 - DT / TAU_S   # 0.98

# "f32r":  PE fast fp32 mode (full speed, ~1e-4 rel err, DMA-bound 4B/elem)
# "f32":   exact fp32 matmul (4x PE cycles, PE-bound)
# "split3": W = fp16 hi + fp8 residual (3B/elem, 3 matmuls, ~3e-5 rel err);
#           25% less HBM traffic and PE-paced DMA (fair arbitration)
W_MODE = "split3"
SCALE_LO = 2.0 ** 16       # split3: fp8 residual pre-scale (exact power of 2)
SCALE_PLO = 2.0 ** 11      # split3: rate-residual pre-scale

# Set by test.py to capture an NTFF profile; LAST_EXEC_NS then holds the
# max-over-cores NEFF execution time of the last run.
TRACE = False
LAST_EXEC_NS = None
LAST_RESULTS = None

_PROGRAM_CACHE = {}


def _install_ntff_hook():
    """Provide antenv.axon_hooks (absent on this image) and register the
    NTFF profile hook exposed by the axon boot shim."""
    if "antenv.axon_hooks" not in sys.modules:
        import antenv

        mod = types.ModuleType("antenv.axon_hooks")
        holder = [None]
        mod.set_axon_ntff_profile_hook = lambda h: holder.__setitem__(0, h)
        mod.get_axon_ntff_profile_hook = lambda: holder[0]
        sys.modules["antenv.axon_hooks"] = mod
        antenv.axon_hooks = mod
    import antenv.axon_hooks as ah

    if ah.get_axon_ntff_profile_hook() is None:
        from trn_agent_boot.trn_boot import _ntff_profile_via_ctypes

        ah.set_axon_ntff_profile_hook(
            _ntff_profile_via_ctypes("/opt/axon/libaxon_pjrt.so")
        )


def _build_program(src_idx: tuple, tgt_idx: tuple, mode: str):
    """Build + bacc-compile the SPMD Bass program (identical on all cores)."""
    f32 = mybir.dt.float32
    f16 = mybir.dt.float16
    f8 = mybir.dt.float8e4
    split3 = mode == "split3"
    wdt = mybir.dt.float32r if mode == "f32r" else f32

    nc = bacc.Bacc("TRN2", target_bir_lowering=False, debug=False,
                   num_devices=N_CORES)

    if split3:
        wh_in = nc.dram_tensor("wh", [128, S, KT, JC], f16,
                               kind="ExternalInput").ap()
        wl_in = nc.dram_tensor("wl", [128, S, KT, JC], f8,
                               kind="ExternalInput").ap()
    else:
        w_in = nc.dram_tensor("w", [128, S, KT, JC], wdt,
                              kind="ExternalInput").ap()
    vt_in = nc.dram_tensor("vt", [128, P, KT], f32, kind="ExternalInput").ap()
    ph_in = nc.dram_tensor("ph", [128, KT], f32, kind="ExternalInput").ap()
    t_in = nc.dram_tensor("t", [1, 1], f32, kind="ExternalInput").ap()
    s_in = nc.dram_tensor("s", [S, JC], f32, kind="ExternalInput").ap()
    v_in = nc.dram_tensor("v", [P, JC], f32, kind="ExternalInput").ap()
    pv_in = nc.dram_tensor("pv", [S, JC], f32, kind="ExternalInput").ap()
    er_in = nc.dram_tensor("er", [S, 1], f32, kind="ExternalInput").ap()
    ms_in = nc.dram_tensor("ms", [S, P], f32, kind="ExternalInput").ap()

    snew_out = nc.dram_tensor("snew", [S, JC], f32, kind="ExternalOutput").ap()
    vnew_out = nc.dram_tensor("vnew", [P, JC], f32, kind="ExternalOutput").ap()
    rates_out = nc.dram_tensor("rates", [P, JC], f32, kind="ExternalOutput").ap()
    errp_out = nc.dram_tensor("errp", [P, 1], f32, kind="ExternalOutput").ap()

    with tile.TileContext(nc) as tc:
        with (
            tc.tile_pool(name="const", bufs=1) as cpool,
            tc.tile_pool(name="wpool", bufs=W_BUFS) as wpool,
            tc.tile_pool(name="stage", bufs=4) as stage_pool,
            # PSUM is 8 banks: split3 uses 3 accum tags x 2 bufs + 2 for the
            # segment sums
            tc.tile_pool(name="dpsum", bufs=2 if split3 else 4,
                         space="PSUM") as dpsum,
            tc.tile_pool(name="apsum", bufs=2, space="PSUM") as apsum,
        ):
            # ---- small resident tiles -------------------------------------
            # all_rates in lhsT layout; for split3 the fp32 rates are split
            # into fp16 hi + scaled fp16 residual + fp8 operand tiles
            rates_sb = cpool.tile([128, P + 1, KT], f32 if split3 else wdt)
            if split3:
                p16_sb = cpool.tile([128, P + 1, KT], f16)
                p16f_sb = cpool.tile([128, P + 1, KT], f32)
                plo_sb = cpool.tile([128, P + 1, KT], f16)
                p8_sb = cpool.tile([128, P + 1, KT], f8)
            vt_sb = cpool.tile([128, P, KT], f32)
            ph_sb = cpool.tile([128, KT], f32)
            t_sb = cpool.tile([128, 1], f32)
            pib_sb = cpool.tile([128, 1], f32)
            pib3_sb = cpool.tile([128, 1], f32)
            sin_sb = cpool.tile([128, KT], f32)
            s1_sb = cpool.tile([128, KT], f32)
            s2_sb = cpool.tile([128, KT], f32)
            # synapse rows split 16/4: the tail group keeps the last
            # synapses' work off the critical path, and separate tiles keep
            # every engine AP starting at partition 0
            SA = 16
            SB = S - SA
            s_a = cpool.tile([SA, JC], f32)
            s_b = cpool.tile([SB, JC], f32)
            v_sb = cpool.tile([P, JC], f32)
            pv_a = cpool.tile([SA, JC], f32)
            pv_b = cpool.tile([SB, JC], f32)
            er_a = cpool.tile([SA, 1], f32)
            er_b = cpool.tile([SB, 1], f32)
            ms_a = cpool.tile([SA, P], f32)
            ms_b = cpool.tile([SB, P], f32)
            snew_a = cpool.tile([SA, JC], f32)
            snew_b = cpool.tile([SB, JC], f32)
            sdec_a = cpool.tile([SA, JC], f32)
            sdec_b = cpool.tile([SB, JC], f32)
            wterm_a = cpool.tile([SA, JC], f32)
            wterm_b = cpool.tile([SB, JC], f32)
            isyn_a = cpool.tile([SA, JC], f32)
            isyn_b = cpool.tile([SB, JC], f32)
            gv_sb = cpool.tile([P, JC], f32)
            dv_sb = cpool.tile([P, JC], f32)
            vnew_sb = cpool.tile([P, JC], f32)
            rates4_sb = cpool.tile([P, JC], f32)
            err_sb = cpool.tile([P, 1], f32)

            # all small input DMAs ride the SWDGE path so the sync HWDGE
            # FIFO carries nothing but the back-to-back W stream
            nc.gpsimd.dma_start(out=vt_sb, in_=vt_in)
            nc.gpsimd.dma_start(out=ph_sb, in_=ph_in)
            nc.gpsimd.dma_start(out=s_a, in_=s_in[0:SA, :])
            nc.gpsimd.dma_start(out=s_b, in_=s_in[SA:S, :])
            nc.gpsimd.dma_start(out=v_sb, in_=v_in)
            nc.gpsimd.dma_start(out=pv_a, in_=pv_in[0:SA, :])
            nc.gpsimd.dma_start(out=pv_b, in_=pv_in[SA:S, :])
            nc.gpsimd.dma_start(out=er_a, in_=er_in[0:SA, :])
            nc.gpsimd.dma_start(out=er_b, in_=er_in[SA:S, :])
            nc.gpsimd.dma_start(out=ms_a, in_=ms_in[0:SA, :])
            nc.gpsimd.dma_start(out=ms_b, in_=ms_in[SA:S, :])
            # broadcast t over the 128 partitions
            nc.gpsimd.dma_start(out=t_sb, in_=t_in.to_broadcast((128, 1)))

            # ---- firing rates of all source populations -------------------
            # rows 0..P-1: sigmoid(v); row P: 0.5*(1+sin(2*pi*f*t + phase))
            nc.scalar.activation(
                out=rates_sb[:, 0:P, :], in_=vt_sb,
                func=mybir.ActivationFunctionType.Sigmoid,
            )
            # ScalarE Sin needs args in [-pi, pi]. a = phase + t_red lies in
            # [0, 4pi) (host pre-reduces the 2*pi*f*t scalar mod 2pi), so
            # subtract 2*pi*k with k = (sign(a-pi) + sign(a-3pi))/2 + 1.
            nc.vector.memset(pib_sb, -float(np.pi))
            nc.vector.memset(pib3_sb, -float(3.0 * np.pi))
            nc.vector.tensor_scalar(
                out=sin_sb, in0=ph_sb, scalar1=t_sb, scalar2=None,
                op0=mybir.AluOpType.add,
            )
            nc.scalar.activation(
                out=s1_sb, in_=sin_sb,
                func=mybir.ActivationFunctionType.Sign, bias=pib_sb,
            )
            nc.scalar.activation(
                out=s2_sb, in_=sin_sb,
                func=mybir.ActivationFunctionType.Sign, bias=pib3_sb,
            )
            nc.vector.tensor_add(s1_sb, s1_sb, s2_sb)
            nc.vector.tensor_scalar(
                out=s1_sb, in0=s1_sb,
                scalar1=-float(np.pi), scalar2=-float(2.0 * np.pi),
                op0=mybir.AluOpType.mult, op1=mybir.AluOpType.add,
            )
            nc.vector.tensor_add(sin_sb, sin_sb, s1_sb)
            nc.scalar.activation(
                out=sin_sb, in_=sin_sb,
                func=mybir.ActivationFunctionType.Sin,
            )
            nc.vector.tensor_scalar(
                out=rates_sb[:, P, :], in0=sin_sb,
                scalar1=0.5, scalar2=0.5,
                op0=mybir.AluOpType.mult, op1=mybir.AluOpType.add,
            )
            if split3:
                # p16 = fp16(rates); plo = (rates - p16) * 2^11 in fp16;
                # p8 = fp8(rates)
                nc.vector.tensor_copy(p16_sb, rates_sb)
                nc.vector.tensor_copy(p16f_sb, p16_sb)
                nc.vector.tensor_sub(p16f_sb, rates_sb, p16f_sb)
                nc.vector.tensor_scalar_mul(plo_sb, p16f_sb, SCALE_PLO)
                nc.vector.tensor_copy(p8_sb, rates_sb)

            # ---- per-synapse drive matmuls + synapse update ---------------
            for s in range(S):
                r = src_idx[s]
                if split3:
                    a_ps = dpsum.tile([1, JC], f32, tag="da")
                    b_ps = dpsum.tile([1, JC], f32, tag="db")
                    c_ps = dpsum.tile([1, JC], f32, tag="dc")
                    for c0 in range(0, KT, CH):
                        wh_tile = wpool.tile([128, CH, JC], f16, tag="wh")
                        wl_tile = wpool.tile([128, CH, JC], f8, tag="wl")
                        nc.sync.dma_start(out=wh_tile,
                                          in_=wh_in[:, s, c0:c0 + CH, :])
                        nc.sync.dma_start(out=wl_tile,
                                          in_=wl_in[:, s, c0:c0 + CH, :])
                        for k in range(CH):
                            kk = c0 + k
                            st, sp = kk == 0, kk == KT - 1
                            nc.tensor.matmul(
                                a_ps, p16_sb[:, r, kk:kk + 1],
                                wh_tile[:, k, :], start=st, stop=sp)
                            nc.tensor.matmul(
                                b_ps, plo_sb[:, r, kk:kk + 1],
                                wh_tile[:, k, :], start=st, stop=sp)
                            nc.tensor.matmul(
                                c_ps, p8_sb[:, r, kk:kk + 1],
                                wl_tile[:, k, :], start=st, stop=sp)
                else:
                    drive_ps = dpsum.tile([1, JC], f32, tag="drive")
                    for c0 in range(0, KT, CH):
                        w_tile = wpool.tile([128, CH, JC], wdt, tag="w")
                        nc.sync.dma_start(out=w_tile,
                                          in_=w_in[:, s, c0:c0 + CH, :])
                        for k in range(CH):
                            kk = c0 + k
                            nc.tensor.matmul(
                                drive_ps, rates_sb[:, r, kk:kk + 1],
                                w_tile[:, k, :],
                                start=(kk == 0), stop=(kk == KT - 1),
                            )
                # s_new partial: DT * drive. ACT stages PSUM -> SBUF at
                # partition 0 (engines can't start mid-partition), then a
                # tiny SBUF->SBUF DMA scatters it to row s.
                stage_sb = stage_pool.tile([1, JC], f32, tag="stage")
                if split3:
                    # drive = a + b/2^11 + c/2^16  (scales fold with DT)
                    tb_sb = stage_pool.tile([1, JC], f32, tag="tb")
                    tc_sb = stage_pool.tile([1, JC], f32, tag="tc")
                    nc.scalar.mul(stage_sb, a_ps, DT)
                    nc.vector.tensor_scalar_mul(tb_sb, b_ps, DT / SCALE_PLO)
                    nc.vector.tensor_scalar_mul(tc_sb, c_ps, DT / SCALE_LO)
                    nc.vector.tensor_add(stage_sb, stage_sb, tb_sb)
                    nc.vector.tensor_add(stage_sb, stage_sb, tc_sb)
                else:
                    nc.scalar.mul(stage_sb, drive_ps, DT)
                # SWDGE path: keeps this dependent little DMA out of the
                # HWDGE FIFOs that stream the W chunks
                if s < SA:
                    nc.gpsimd.dma_start(out=snew_a[s:s + 1, :], in_=stage_sb)
                else:
                    nc.gpsimd.dma_start(out=snew_b[s - SA:s - SA + 1, :],
                                        in_=stage_sb)

            # ---- synapse state + currents (two row groups; group a runs
            # while the last synapses are still streaming) -------------------
            nc.vector.tensor_scalar_mul(sdec_a, s_a, DECAY)
            nc.vector.tensor_scalar_mul(sdec_b, s_b, DECAY)
            # wterm = E_rev - post_v
            nc.vector.tensor_scalar(
                out=wterm_a, in0=pv_a, scalar1=-1.0, scalar2=er_a,
                op0=mybir.AluOpType.mult, op1=mybir.AluOpType.add,
            )
            nc.vector.tensor_scalar(
                out=wterm_b, in0=pv_b, scalar1=-1.0, scalar2=er_b,
                op0=mybir.AluOpType.mult, op1=mybir.AluOpType.add,
            )
            itot_ps = apsum.tile([P, JC], f32, tag="acc")
            gtot_ps = apsum.tile([P, JC], f32, tag="acc")

            nc.vector.tensor_add(snew_a, snew_a, sdec_a)
            nc.sync.dma_start(out=snew_out[0:SA, :], in_=snew_a)
            nc.vector.tensor_mul(isyn_a, snew_a, wterm_a)
            nc.tensor.matmul(itot_ps, ms_a, isyn_a, start=True, stop=False)
            nc.tensor.matmul(gtot_ps, ms_a, snew_a, start=True, stop=False)

            nc.vector.tensor_add(snew_b, snew_b, sdec_b)
            nc.sync.dma_start(out=snew_out[SA:S, :], in_=snew_b)
            nc.vector.tensor_mul(isyn_b, snew_b, wterm_b)
            nc.tensor.matmul(itot_ps, ms_b, isyn_b, start=False, stop=True)
            nc.tensor.matmul(gtot_ps, ms_b, snew_b, start=False, stop=True)

            # ---- population integration -----------------------------------
            nc.vector.tensor_mul(gv_sb, gtot_ps, v_sb)
            nc.vector.tensor_sub(dv_sb, itot_ps, gv_sb)
            nc.vector.tensor_sub(dv_sb, dv_sb, v_sb)
            nc.vector.tensor_scalar_mul(dv_sb, dv_sb, 1.0 / TAU_M)
            nc.vector.tensor_reduce(
                out=err_sb, in_=dv_sb, axis=mybir.AxisListType.X,
                op=mybir.AluOpType.add, apply_absolute_value=True,
            )
            nc.vector.tensor_scalar_mul(vnew_sb, dv_sb, DT)
            nc.vector.tensor_add(vnew_sb, vnew_sb, v_sb)
            nc.scalar.activation(
                out=rates4_sb, in_=vnew_sb,
                func=mybir.ActivationFunctionType.Sigmoid,
            )

            nc.sync.dma_start(out=vnew_out, in_=vnew_sb)
            nc.sync.dma_start(out=rates_out, in_=rates4_sb)
            nc.sync.dma_start(out=errp_out, in_=err_sb)

    nc.compile()
    return nc


def _get_program(src_idx, tgt_idx, mode):
    key = (tuple(int(x) for x in src_idx), tuple(int(x) for x in tgt_idx), mode)
    if key not in _PROGRAM_CACHE:
        _PROGRAM_CACHE[key] = _build_program(key[0], key[1], mode)
    return _PROGRAM_CACHE[key]


def _prep_inputs(t, v, s, W, E_rev, phase, src_idx, tgt_idx, mode=None):
    """Host-side shard/layout prep (gathers/transposes/precision encode)."""
    import ml_dtypes

    mode = mode or W_MODE
    f32 = np.float32
    v2 = np.asarray(v, f32)[:, 0, :]                      # [P, N]
    s2 = np.asarray(s, f32)[:, 0, :]                      # [S, N]
    W = np.asarray(W, f32)
    tgt = np.asarray(tgt_idx)

    # lhsT layouts: element [p, ..., k] = x[..., k*128 + p]
    vt = np.ascontiguousarray(v2.reshape(P, KT, 128).transpose(2, 0, 1))
    ph = np.ascontiguousarray(
        np.asarray(phase, f32).reshape(KT, 128).transpose(1, 0))
    msel = np.zeros((S, P), f32)
    msel[np.arange(S), tgt] = 1.0
    erev = np.asarray(E_rev, f32).reshape(S, 1)
    # t enters as the fp32 angle 2*pi*f*t, pre-reduced mod 2pi (one host
    # scalar op) so the on-device sin range reduction only spans [0, 4pi).
    tr = f32(2.0 * np.pi * IN_FREQ) * np.asarray(t, f32).reshape(-1)[0]
    if tr >= f32(2.0 * np.pi) or tr < 0.0:
        tr = f32(np.float64(tr) % (2.0 * np.pi))
    t_arr = np.asarray(tr, f32).reshape(1, 1)

    # W[s, i, j] with i = kt*128 + p, j = c*JC + jj  ->  per-core [128, S, KT, JC]
    if mode == "split3":
        WH = W.astype(np.float16)
        WL = ((W - WH.astype(f32)) * f32(SCALE_LO)).astype(ml_dtypes.float8_e4m3)
        WH5 = WH.reshape(S, KT, 128, N_CORES, JC)
        WL5 = WL.reshape(S, KT, 128, N_CORES, JC)
    else:
        W5 = W.reshape(S, KT, 128, N_CORES, JC)

    in_maps = []
    for c in range(N_CORES):
        sl = slice(c * JC, (c + 1) * JC)
        vc = np.ascontiguousarray(v2[:, sl])
        m = dict(
            vt=vt, ph=ph, t=t_arr,
            s=np.ascontiguousarray(s2[:, sl]),
            v=vc,
            pv=np.ascontiguousarray(vc[tgt]),
            er=erev, ms=msel,
        )
        if mode == "split3":
            m["wh"] = np.ascontiguousarray(WH5[:, :, :, c, :].transpose(2, 0, 1, 3))
            m["wl"] = np.ascontiguousarray(WL5[:, :, :, c, :].transpose(2, 0, 1, 3))
        else:
            m["w"] = np.ascontiguousarray(W5[:, :, :, c, :].transpose(2, 0, 1, 3))
        in_maps.append(m)
    return in_maps


def kernel(t, v, s, W, E_rev, phase, src_idx, tgt_idx):
    global LAST_EXEC_NS, LAST_RESULTS
    nc = _get_program(src_idx, tgt_idx, W_MODE)
    in_maps = _prep_inputs(t, v, s, W, E_rev, phase, src_idx, tgt_idx)

    kwargs = {}
    if TRACE:
        _install_ntff_hook()
        kwargs = dict(trace=True, trace_cores=list(range(N_CORES)))
    res = run_bass_kernel_spmd(nc, in_maps, core_ids=list(range(N_CORES)),
                               **kwargs)
    LAST_EXEC_NS = res.exec_time_ns
    LAST_RESULTS = res

    # ---- unshard -----------------------------------------------------------
    f32 = np.float32
    s_new = np.empty((S, B, N), f32)
    v_new = np.empty((P, B, N), f32)
    rates = np.empty((P, N), f32)
    err_acc = 0.0
    for c in range(N_CORES):
        r = res.results[c]
        sl = slice(c * JC, (c + 1) * JC)
        s_new[:, 0, sl] = r["snew"]
        v_new[:, 0, sl] = r["vnew"]
        rates[:, sl] = r["rates"]
        err_acc += float(r["errp"].sum())
    err = np.asarray(err_acc / N, f32)
    out = np.concatenate(
        [rates.reshape(1, P * N), err.reshape(1, 1)], axis=1).astype(f32)
    return out, v_new, s_new


# revision 28
# speedup vs baseline: 1.3346x; 1.1266x over previous
"""Trainium2 Bass kernel for nn_NetworkRNNCell (gnn message passing).

Contract: kernel(**inputs) takes FULL unsharded numpy inputs (as produced by
setup_inputs()) and returns the FULL output tuple (out, v_new, s_new),
matching reference() exactly in shapes/dtypes.

Sharding: the unit axis N (=4096) of the synaptic weight matrices W[S,N,N]
is split column-wise across the 8 NeuronCores (512 output columns per core).
Every core processes all S=20 synapses for its column slice:
  - drive[s, j] = sum_i pre[s,i] * W[s,i,j]   (PE matmuls, i tiled by 128)
  - synapse update, conductance currents, segment-sum onto the P=4 target
    populations (one tiny 0/1 selection-matrix matmul), Euler integration,
    output rates -- all elementwise over the local j slice.
This needs no cross-core collectives: the segment-sum runs over the synapse
axis which stays fully local. The only global reduction is the scalar
stability error; each core emits 4 partial sums which the host combines
during unshard.
"""

import sys
import types

import numpy as np

import concourse.bacc as bacc
import concourse.bass as bass
import concourse.tile as tile
from concourse import mybir
from concourse.bass_utils import run_bass_kernel_spmd

# network constants
DT = 0.1
TAU_M = 10.0
TAU_S = 5.0
P = 4
S = 20
N = 4096
B = 1
IN_FREQ = 0.008

N_CORES = 8
JC = N // N_CORES          # output columns per core (512)
KT = N // 128              # contraction tiles of 128 (32)
CH = 8                     # k-tiles per W DMA chunk (8 -> 2 MB chunks)
W_BUFS = 6                 # W tile buffering depth

DECAY = 1.0 - DT / TAU_S   # 0.98

# "f32r":  PE fast fp32 mode (full speed, ~1e-4 rel err, DMA-bound 4B/elem)
# "f32":   exact fp32 matmul (4x PE cycles, PE-bound)
# "split3": W = fp16 hi + fp8 residual (3B/elem, 3 matmuls, ~3e-5 rel err);
#           25% less HBM traffic and PE-paced DMA (fair arbitration)
W_MODE = "split3"
SCALE_LO = 2.0 ** 16       # split3: fp8 residual pre-scale (exact power of 2)
SCALE_PLO = 2.0 ** 11      # split3: rate-residual pre-scale

# Set by test.py to capture an NTFF profile; LAST_EXEC_NS then holds the
# max-over-cores NEFF execution time of the last run.
TRACE = False
LAST_EXEC_NS = None
LAST_RESULTS = None

_PROGRAM_CACHE = {}


def _install_ntff_hook():
    """Provide antenv.axon_hooks (absent on this image) and register the
    NTFF profile hook exposed by the axon boot shim."""
    if "antenv.axon_hooks" not in sys.modules:
        import antenv

        mod = types.ModuleType("antenv.axon_hooks")
        holder = [None]
        mod.set_axon_ntff_profile_hook = lambda h: holder.__setitem__(0, h)
        mod.get_axon_ntff_profile_hook = lambda: holder[0]
        sys.modules["antenv.axon_hooks"] = mod
        antenv.axon_hooks = mod
    import antenv.axon_hooks as ah

    if ah.get_axon_ntff_profile_hook() is None:
        from trn_agent_boot.trn_boot import _ntff_profile_via_ctypes

        ah.set_axon_ntff_profile_hook(
            _ntff_profile_via_ctypes("/opt/axon/libaxon_pjrt.so")
        )


def _build_program(src_idx: tuple, tgt_idx: tuple, mode: str):
    """Build + bacc-compile the SPMD Bass program (identical on all cores)."""
    f32 = mybir.dt.float32
    f16 = mybir.dt.float16
    f8 = mybir.dt.float8e4
    split3 = mode == "split3"
    wdt = mybir.dt.float32r if mode == "f32r" else f32

    nc = bacc.Bacc("TRN2", target_bir_lowering=False, debug=False,
                   num_devices=N_CORES)

    if split3:
        wh_in = nc.dram_tensor("wh", [128, S, KT, JC], f16,
                               kind="ExternalInput").ap()
        wl_in = nc.dram_tensor("wl", [128, S, KT, JC], f8,
                               kind="ExternalInput").ap()
    else:
        w_in = nc.dram_tensor("w", [128, S, KT, JC], wdt,
                              kind="ExternalInput").ap()
    vt_in = nc.dram_tensor("vt", [128, P, KT], f32, kind="ExternalInput").ap()
    ph_in = nc.dram_tensor("ph", [128, KT], f32, kind="ExternalInput").ap()
    t_in = nc.dram_tensor("t", [1, 1], f32, kind="ExternalInput").ap()
    s_in = nc.dram_tensor("s", [S, JC], f32, kind="ExternalInput").ap()
    v_in = nc.dram_tensor("v", [P, JC], f32, kind="ExternalInput").ap()
    pv_in = nc.dram_tensor("pv", [S, JC], f32, kind="ExternalInput").ap()
    er_in = nc.dram_tensor("er", [S, 1], f32, kind="ExternalInput").ap()
    ms_in = nc.dram_tensor("ms", [S, P], f32, kind="ExternalInput").ap()

    snew_out = nc.dram_tensor("snew", [S, JC], f32, kind="ExternalOutput").ap()
    vnew_out = nc.dram_tensor("vnew", [P, JC], f32, kind="ExternalOutput").ap()
    rates_out = nc.dram_tensor("rates", [P, JC], f32, kind="ExternalOutput").ap()
    errp_out = nc.dram_tensor("errp", [P, 1], f32, kind="ExternalOutput").ap()

    with tile.TileContext(nc) as tc:
        with (
            tc.tile_pool(name="const", bufs=1) as cpool,
            tc.tile_pool(name="wpool", bufs=W_BUFS) as wpool,
            tc.tile_pool(name="stage", bufs=4) as stage_pool,
            # PSUM is 8 banks: split3 uses 3 accum tags x 2 bufs + 2 for the
            # segment sums
            tc.tile_pool(name="dpsum", bufs=2 if split3 else 4,
                         space="PSUM") as dpsum,
            tc.tile_pool(name="apsum", bufs=2, space="PSUM") as apsum,
        ):
            # ---- small resident tiles -------------------------------------
            # all_rates in lhsT layout; for split3 the fp32 rates are split
            # into fp16 hi + scaled fp16 residual + fp8 operand tiles
            rates_sb = cpool.tile([128, P + 1, KT], f32 if split3 else wdt)
            if split3:
                p16_sb = cpool.tile([128, P + 1, KT], f16)
                p16f_sb = cpool.tile([128, P + 1, KT], f32)
                plo_sb = cpool.tile([128, P + 1, KT], f16)
                p8_sb = cpool.tile([128, P + 1, KT], f8)
            vt_sb = cpool.tile([128, P, KT], f32)
            ph_sb = cpool.tile([128, KT], f32)
            t_sb = cpool.tile([128, 1], f32)
            pib_sb = cpool.tile([128, 1], f32)
            pib3_sb = cpool.tile([128, 1], f32)
            sin_sb = cpool.tile([128, KT], f32)
            s1_sb = cpool.tile([128, KT], f32)
            s2_sb = cpool.tile([128, KT], f32)
            # synapse rows split 16/4: the tail group keeps the last
            # synapses' work off the critical path, and separate tiles keep
            # every engine AP starting at partition 0
            SA = 16
            SB = S - SA
            s_a = cpool.tile([SA, JC], f32)
            s_b = cpool.tile([SB, JC], f32)
            v_sb = cpool.tile([P, JC], f32)
            pv_a = cpool.tile([SA, JC], f32)
            pv_b = cpool.tile([SB, JC], f32)
            er_a = cpool.tile([SA, 1], f32)
            er_b = cpool.tile([SB, 1], f32)
            ms_a = cpool.tile([SA, P], f32)
            ms_b = cpool.tile([SB, P], f32)
            snew_a = cpool.tile([SA, JC], f32)
            snew_b = cpool.tile([SB, JC], f32)
            sdec_a = cpool.tile([SA, JC], f32)
            sdec_b = cpool.tile([SB, JC], f32)
            wterm_a = cpool.tile([SA, JC], f32)
            wterm_b = cpool.tile([SB, JC], f32)
            isyn_a = cpool.tile([SA, JC], f32)
            isyn_b = cpool.tile([SB, JC], f32)
            gv_sb = cpool.tile([P, JC], f32)
            dv_sb = cpool.tile([P, JC], f32)
            vnew_sb = cpool.tile([P, JC], f32)
            rates4_sb = cpool.tile([P, JC], f32)
            err_sb = cpool.tile([P, 1], f32)

            # rate-prep inputs gate the first matmul: issue them first on the
            # fast HWDGE ring (ahead of the W stream). Everything else rides
            # SWDGE so the sync FIFO stays a back-to-back W stream.
            nc.sync.dma_start(out=vt_sb, in_=vt_in)
            nc.sync.dma_start(out=ph_sb, in_=ph_in)
            nc.gpsimd.dma_start(out=t_sb, in_=t_in.to_broadcast((128, 1)))
            nc.gpsimd.dma_start(out=s_a, in_=s_in[0:SA, :])
            nc.gpsimd.dma_start(out=s_b, in_=s_in[SA:S, :])
            nc.gpsimd.dma_start(out=v_sb, in_=v_in)
            nc.gpsimd.dma_start(out=pv_a, in_=pv_in[0:SA, :])
            nc.gpsimd.dma_start(out=pv_b, in_=pv_in[SA:S, :])
            nc.gpsimd.dma_start(out=er_a, in_=er_in[0:SA, :])
            nc.gpsimd.dma_start(out=er_b, in_=er_in[SA:S, :])
            nc.gpsimd.dma_start(out=ms_a, in_=ms_in[0:SA, :])
            nc.gpsimd.dma_start(out=ms_b, in_=ms_in[SA:S, :])

            # ---- firing rates of all source populations -------------------
            # rows 0..P-1: sigmoid(v); row P: 0.5*(1+sin(2*pi*f*t + phase))
            nc.scalar.activation(
                out=rates_sb[:, 0:P, :], in_=vt_sb,
                func=mybir.ActivationFunctionType.Sigmoid,
            )
            # ScalarE Sin needs args in [-pi, pi]. a = phase + t_red lies in
            # [0, 4pi) (host pre-reduces the 2*pi*f*t scalar mod 2pi), so
            # subtract 2*pi*k with k = (sign(a-pi) + sign(a-3pi))/2 + 1.
            nc.vector.memset(pib_sb, -float(np.pi))
            nc.vector.memset(pib3_sb, -float(3.0 * np.pi))
            nc.vector.tensor_scalar(
                out=sin_sb, in0=ph_sb, scalar1=t_sb, scalar2=None,
                op0=mybir.AluOpType.add,
            )
            nc.scalar.activation(
                out=s1_sb, in_=sin_sb,
                func=mybir.ActivationFunctionType.Sign, bias=pib_sb,
            )
            nc.scalar.activation(
                out=s2_sb, in_=sin_sb,
                func=mybir.ActivationFunctionType.Sign, bias=pib3_sb,
            )
            nc.vector.tensor_add(s1_sb, s1_sb, s2_sb)
            nc.vector.tensor_scalar(
                out=s1_sb, in0=s1_sb,
                scalar1=-float(np.pi), scalar2=-float(2.0 * np.pi),
                op0=mybir.AluOpType.mult, op1=mybir.AluOpType.add,
            )
            nc.vector.tensor_add(sin_sb, sin_sb, s1_sb)
            nc.scalar.activation(
                out=sin_sb, in_=sin_sb,
                func=mybir.ActivationFunctionType.Sin,
            )
            nc.vector.tensor_scalar(
                out=rates_sb[:, P, :], in0=sin_sb,
                scalar1=0.5, scalar2=0.5,
                op0=mybir.AluOpType.mult, op1=mybir.AluOpType.add,
            )
            if split3:
                # p16 = fp16(rates); plo = (rates - p16) * 2^11 in fp16;
                # p8 = fp8(rates)
                nc.vector.tensor_copy(p16_sb, rates_sb)
                nc.vector.tensor_copy(p16f_sb, p16_sb)
                nc.vector.tensor_sub(p16f_sb, rates_sb, p16f_sb)
                nc.vector.tensor_scalar_mul(plo_sb, p16f_sb, SCALE_PLO)
                nc.vector.tensor_copy(p8_sb, rates_sb)

            # ---- per-synapse drive matmuls + synapse update ---------------
            for s in range(S):
                r = src_idx[s]
                if split3:
                    # one PSUM bank holds all three accumulators: the three
                    # streams run in PE column groups 0/1/2 (outputs at
                    # partitions 0/32/64) so their LDWEIGHTS/MATMUL pairs
                    # pipeline instead of serializing on one PE column
                    abc_ps = dpsum.tile([128, JC], f32, tag="dabc")
                    a_ps = abc_ps[0:1, :]
                    b_ps = abc_ps[32:33, :]
                    c_ps = abc_ps[64:65, :]
                    for c0 in range(0, KT, CH):
                        wh_tile = wpool.tile([128, CH, JC], f16, tag="wh")
                        wl_tile = wpool.tile([128, CH, JC], f8, tag="wl")
                        nc.sync.dma_start(out=wh_tile,
                                          in_=wh_in[:, s, c0:c0 + CH, :])
                        nc.sync.dma_start(out=wl_tile,
                                          in_=wl_in[:, s, c0:c0 + CH, :])
                        for k in range(CH):
                            kk = c0 + k
                            st, sp = kk == 0, kk == KT - 1
                            nc.tensor.matmul(
                                a_ps, p16_sb[:, r, kk:kk + 1],
                                wh_tile[:, k, :], start=st, stop=sp,
                                tile_position=(0, 0))
                            nc.tensor.matmul(
                                b_ps, plo_sb[:, r, kk:kk + 1],
                                wh_tile[:, k, :], start=st, stop=sp,
                                tile_position=(0, 32))
                            nc.tensor.matmul(
                                c_ps, p8_sb[:, r, kk:kk + 1],
                                wl_tile[:, k, :], start=st, stop=sp,
                                tile_position=(0, 64))
                else:
                    drive_ps = dpsum.tile([1, JC], f32, tag="drive")
                    for c0 in range(0, KT, CH):
                        w_tile = wpool.tile([128, CH, JC], wdt, tag="w")
                        nc.sync.dma_start(out=w_tile,
                                          in_=w_in[:, s, c0:c0 + CH, :])
                        for k in range(CH):
                            kk = c0 + k
                            nc.tensor.matmul(
                                drive_ps, rates_sb[:, r, kk:kk + 1],
                                w_tile[:, k, :],
                                start=(kk == 0), stop=(kk == KT - 1),
                            )
                # s_new partial: DT * drive. ACT stages PSUM -> SBUF at
                # partition 0 (engines can't start mid-partition), then a
                # tiny SBUF->SBUF DMA scatters it to row s.
                stage_sb = stage_pool.tile([1, JC], f32, tag="stage")
                if split3:
                    # drive = a + b/2^11 + c/2^16  (scales fold with DT)
                    tb_sb = stage_pool.tile([1, JC], f32, tag="tb")
                    tc_sb = stage_pool.tile([1, JC], f32, tag="tc")
                    nc.scalar.mul(stage_sb, a_ps, DT)
                    nc.vector.tensor_scalar_mul(tb_sb, b_ps, DT / SCALE_PLO)
                    nc.vector.tensor_scalar_mul(tc_sb, c_ps, DT / SCALE_LO)
                    nc.vector.tensor_add(stage_sb, stage_sb, tb_sb)
                    nc.vector.tensor_add(stage_sb, stage_sb, tc_sb)
                else:
                    nc.scalar.mul(stage_sb, drive_ps, DT)
                # SWDGE path: keeps this dependent little DMA out of the
                # HWDGE FIFOs that stream the W chunks
                if s < SA:
                    nc.gpsimd.dma_start(out=snew_a[s:s + 1, :], in_=stage_sb)
                else:
                    nc.gpsimd.dma_start(out=snew_b[s - SA:s - SA + 1, :],
                                        in_=stage_sb)

            # ---- synapse state + currents (two row groups; group a runs
            # while the last synapses are still streaming) -------------------
            nc.vector.tensor_scalar_mul(sdec_a, s_a, DECAY)
            nc.vector.tensor_scalar_mul(sdec_b, s_b, DECAY)
            # wterm = E_rev - post_v
            nc.vector.tensor_scalar(
                out=wterm_a, in0=pv_a, scalar1=-1.0, scalar2=er_a,
                op0=mybir.AluOpType.mult, op1=mybir.AluOpType.add,
            )
            nc.vector.tensor_scalar(
                out=wterm_b, in0=pv_b, scalar1=-1.0, scalar2=er_b,
                op0=mybir.AluOpType.mult, op1=mybir.AluOpType.add,
            )
            itot_ps = apsum.tile([P, JC], f32, tag="acc")
            gtot_ps = apsum.tile([P, JC], f32, tag="acc")

            nc.vector.tensor_add(snew_a, snew_a, sdec_a)
            nc.sync.dma_start(out=snew_out[0:SA, :], in_=snew_a)
            nc.vector.tensor_mul(isyn_a, snew_a, wterm_a)
            nc.tensor.matmul(itot_ps, ms_a, isyn_a, start=True, stop=False)
            nc.tensor.matmul(gtot_ps, ms_a, snew_a, start=True, stop=False)

            nc.vector.tensor_add(snew_b, snew_b, sdec_b)
            nc.sync.dma_start(out=snew_out[SA:S, :], in_=snew_b)
            nc.vector.tensor_mul(isyn_b, snew_b, wterm_b)
            nc.tensor.matmul(itot_ps, ms_b, isyn_b, start=False, stop=True)
            nc.tensor.matmul(gtot_ps, ms_b, snew_b, start=False, stop=True)

            # ---- population integration -----------------------------------
            nc.vector.tensor_mul(gv_sb, gtot_ps, v_sb)
            nc.vector.tensor_sub(dv_sb, itot_ps, gv_sb)
            nc.vector.tensor_sub(dv_sb, dv_sb, v_sb)
            nc.vector.tensor_scalar_mul(dv_sb, dv_sb, 1.0 / TAU_M)
            nc.vector.tensor_reduce(
                out=err_sb, in_=dv_sb, axis=mybir.AxisListType.X,
                op=mybir.AluOpType.add, apply_absolute_value=True,
            )
            nc.vector.tensor_scalar_mul(vnew_sb, dv_sb, DT)
            nc.vector.tensor_add(vnew_sb, vnew_sb, v_sb)
            nc.scalar.activation(
                out=rates4_sb, in_=vnew_sb,
                func=mybir.ActivationFunctionType.Sigmoid,
            )

            nc.sync.dma_start(out=vnew_out, in_=vnew_sb)
            nc.sync.dma_start(out=rates_out, in_=rates4_sb)
            nc.sync.dma_start(out=errp_out, in_=err_sb)

    nc.compile()
    return nc


def _get_program(src_idx, tgt_idx, mode):
    key = (tuple(int(x) for x in src_idx), tuple(int(x) for x in tgt_idx), mode)
    if key not in _PROGRAM_CACHE:
        _PROGRAM_CACHE[key] = _build_program(key[0], key[1], mode)
    return _PROGRAM_CACHE[key]


def _prep_inputs(t, v, s, W, E_rev, phase, src_idx, tgt_idx, mode=None):
    """Host-side shard/layout prep (gathers/transposes/precision encode)."""
    import ml_dtypes

    mode = mode or W_MODE
    f32 = np.float32
    v2 = np.asarray(v, f32)[:, 0, :]                      # [P, N]
    s2 = np.asarray(s, f32)[:, 0, :]                      # [S, N]
    W = np.asarray(W, f32)
    tgt = np.asarray(tgt_idx)

    # lhsT layouts: element [p, ..., k] = x[..., k*128 + p]
    vt = np.ascontiguousarray(v2.reshape(P, KT, 128).transpose(2, 0, 1))
    ph = np.ascontiguousarray(
        np.asarray(phase, f32).reshape(KT, 128).transpose(1, 0))
    msel = np.zeros((S, P), f32)
    msel[np.arange(S), tgt] = 1.0
    erev = np.asarray(E_rev, f32).reshape(S, 1)
    # t enters as the fp32 angle 2*pi*f*t, pre-reduced mod 2pi (one host
    # scalar op) so the on-device sin range reduction only spans [0, 4pi).
    tr = f32(2.0 * np.pi * IN_FREQ) * np.asarray(t, f32).reshape(-1)[0]
    if tr >= f32(2.0 * np.pi) or tr < 0.0:
        tr = f32(np.float64(tr) % (2.0 * np.pi))
    t_arr = np.asarray(tr, f32).reshape(1, 1)

    # W[s, i, j] with i = kt*128 + p, j = c*JC + jj  ->  per-core [128, S, KT, JC]
    if mode == "split3":
        WH = W.astype(np.float16)
        WL = ((W - WH.astype(f32)) * f32(SCALE_LO)).astype(ml_dtypes.float8_e4m3)
        WH5 = WH.reshape(S, KT, 128, N_CORES, JC)
        WL5 = WL.reshape(S, KT, 128, N_CORES, JC)
    else:
        W5 = W.reshape(S, KT, 128, N_CORES, JC)

    in_maps = []
    for c in range(N_CORES):
        sl = slice(c * JC, (c + 1) * JC)
        vc = np.ascontiguousarray(v2[:, sl])
        m = dict(
            vt=vt, ph=ph, t=t_arr,
            s=np.ascontiguousarray(s2[:, sl]),
            v=vc,
            pv=np.ascontiguousarray(vc[tgt]),
            er=erev, ms=msel,
        )
        if mode == "split3":
            m["wh"] = np.ascontiguousarray(WH5[:, :, :, c, :].transpose(2, 0, 1, 3))
            m["wl"] = np.ascontiguousarray(WL5[:, :, :, c, :].transpose(2, 0, 1, 3))
        else:
            m["w"] = np.ascontiguousarray(W5[:, :, :, c, :].transpose(2, 0, 1, 3))
        in_maps.append(m)
    return in_maps


def kernel(t, v, s, W, E_rev, phase, src_idx, tgt_idx):
    global LAST_EXEC_NS, LAST_RESULTS
    nc = _get_program(src_idx, tgt_idx, W_MODE)
    in_maps = _prep_inputs(t, v, s, W, E_rev, phase, src_idx, tgt_idx)

    kwargs = {}
    if TRACE:
        _install_ntff_hook()
        kwargs = dict(trace=True, trace_cores=list(range(N_CORES)))
    res = run_bass_kernel_spmd(nc, in_maps, core_ids=list(range(N_CORES)),
                               **kwargs)
    LAST_EXEC_NS = res.exec_time_ns
    LAST_RESULTS = res

    # ---- unshard -----------------------------------------------------------
    f32 = np.float32
    s_new = np.empty((S, B, N), f32)
    v_new = np.empty((P, B, N), f32)
    rates = np.empty((P, N), f32)
    err_acc = 0.0
    for c in range(N_CORES):
        r = res.results[c]
        sl = slice(c * JC, (c + 1) * JC)
        s_new[:, 0, sl] = r["snew"]
        v_new[:, 0, sl] = r["vnew"]
        rates[:, sl] = r["rates"]
        err_acc += float(r["errp"].sum())
    err = np.asarray(err_acc / N, f32)
    out = np.concatenate(
        [rates.reshape(1, P * N), err.reshape(1, 1)], axis=1).astype(f32)
    return out, v_new, s_new


# revision 31
# speedup vs baseline: 1.3382x; 1.0027x over previous
"""Trainium2 Bass kernel for nn_NetworkRNNCell (gnn message passing).

Contract: kernel(**inputs) takes FULL unsharded numpy inputs (as produced by
setup_inputs()) and returns the FULL output tuple (out, v_new, s_new),
matching reference() exactly in shapes/dtypes.

Sharding: the unit axis N (=4096) of the synaptic weight matrices W[S,N,N]
is split column-wise across the 8 NeuronCores (512 output columns per core).
Every core processes all S=20 synapses for its column slice:
  - drive[s, j] = sum_i pre[s,i] * W[s,i,j]   (PE matmuls, i tiled by 128)
  - synapse update, conductance currents, segment-sum onto the P=4 target
    populations (one tiny 0/1 selection-matrix matmul), Euler integration,
    output rates -- all elementwise over the local j slice.
This needs no cross-core collectives: the segment-sum runs over the synapse
axis which stays fully local. The only global reduction is the scalar
stability error; each core emits 4 partial sums which the host combines
during unshard.
"""

import sys
import types

import numpy as np

import concourse.bacc as bacc
import concourse.bass as bass
import concourse.tile as tile
from concourse import mybir
from concourse.bass_utils import run_bass_kernel_spmd

# network constants
DT = 0.1
TAU_M = 10.0
TAU_S = 5.0
P = 4
S = 20
N = 4096
B = 1
IN_FREQ = 0.008

N_CORES = 8
JC = N // N_CORES          # output columns per core (512)
KT = N // 128              # contraction tiles of 128 (32)
CH = 8                     # k-tiles per W DMA chunk (8 -> 2 MB chunks)
W_BUFS = 8                 # W tile buffering depth

DECAY = 1.0 - DT / TAU_S   # 0.98

# "f32r":  PE fast fp32 mode (full speed, ~1e-4 rel err, DMA-bound 4B/elem)
# "f32":   exact fp32 matmul (4x PE cycles, PE-bound)
# "split3": W = fp16 hi + fp8 residual (3B/elem, 3 matmuls, ~3e-5 rel err);
#           25% less HBM traffic and PE-paced DMA (fair arbitration)
W_MODE = "split3"
SCALE_LO = 2.0 ** 16       # split3: fp8 residual pre-scale (exact power of 2)
SCALE_PLO = 2.0 ** 11      # split3: rate-residual pre-scale

# Set by test.py to capture an NTFF profile; LAST_EXEC_NS then holds the
# max-over-cores NEFF execution time of the last run.
TRACE = False
LAST_EXEC_NS = None
LAST_RESULTS = None

_PROGRAM_CACHE = {}


def _install_ntff_hook():
    """Provide antenv.axon_hooks (absent on this image) and register the
    NTFF profile hook exposed by the axon boot shim."""
    if "antenv.axon_hooks" not in sys.modules:
        import antenv

        mod = types.ModuleType("antenv.axon_hooks")
        holder = [None]
        mod.set_axon_ntff_profile_hook = lambda h: holder.__setitem__(0, h)
        mod.get_axon_ntff_profile_hook = lambda: holder[0]
        sys.modules["antenv.axon_hooks"] = mod
        antenv.axon_hooks = mod
    import antenv.axon_hooks as ah

    if ah.get_axon_ntff_profile_hook() is None:
        from trn_agent_boot.trn_boot import _ntff_profile_via_ctypes

        ah.set_axon_ntff_profile_hook(
            _ntff_profile_via_ctypes("/opt/axon/libaxon_pjrt.so")
        )


def _build_program(src_idx: tuple, tgt_idx: tuple, mode: str):
    """Build + bacc-compile the SPMD Bass program (identical on all cores)."""
    f32 = mybir.dt.float32
    f16 = mybir.dt.float16
    f8 = mybir.dt.float8e4
    split3 = mode == "split3"
    wdt = mybir.dt.float32r if mode == "f32r" else f32

    nc = bacc.Bacc("TRN2", target_bir_lowering=False, debug=False,
                   num_devices=N_CORES)

    if split3:
        wh_in = nc.dram_tensor("wh", [128, S, KT, JC], f16,
                               kind="ExternalInput").ap()
        wl_in = nc.dram_tensor("wl", [128, S, KT, JC], f8,
                               kind="ExternalInput").ap()
    else:
        w_in = nc.dram_tensor("w", [128, S, KT, JC], wdt,
                              kind="ExternalInput").ap()
    vt_in = nc.dram_tensor("vt", [128, P, KT], f32, kind="ExternalInput").ap()
    ph_in = nc.dram_tensor("ph", [128, KT], f32, kind="ExternalInput").ap()
    t_in = nc.dram_tensor("t", [1, 1], f32, kind="ExternalInput").ap()
    s_in = nc.dram_tensor("s", [S, JC], f32, kind="ExternalInput").ap()
    v_in = nc.dram_tensor("v", [P, JC], f32, kind="ExternalInput").ap()
    pv_in = nc.dram_tensor("pv", [S, JC], f32, kind="ExternalInput").ap()
    er_in = nc.dram_tensor("er", [S, 1], f32, kind="ExternalInput").ap()
    ms_in = nc.dram_tensor("ms", [S, P], f32, kind="ExternalInput").ap()

    snew_out = nc.dram_tensor("snew", [S, JC], f32, kind="ExternalOutput").ap()
    vnew_out = nc.dram_tensor("vnew", [P, JC], f32, kind="ExternalOutput").ap()
    rates_out = nc.dram_tensor("rates", [P, JC], f32, kind="ExternalOutput").ap()
    errp_out = nc.dram_tensor("errp", [P, 1], f32, kind="ExternalOutput").ap()

    with tile.TileContext(nc) as tc:
        with (
            tc.tile_pool(name="const", bufs=1) as cpool,
            tc.tile_pool(name="wpool", bufs=W_BUFS) as wpool,
            tc.tile_pool(name="stage", bufs=4) as stage_pool,
            # PSUM is 8 banks: split3 packs its 3 accumulators into one bank
            # (partitions 0/32/64) x 4 bufs, + 2 banks for the segment sums
            tc.tile_pool(name="dpsum", bufs=4,
                         space="PSUM") as dpsum,
            tc.tile_pool(name="apsum", bufs=2, space="PSUM") as apsum,
        ):
            # ---- small resident tiles -------------------------------------
            # all_rates in lhsT layout; for split3 the fp32 rates are split
            # into fp16 hi + scaled fp16 residual + fp8 operand tiles
            rates_sb = cpool.tile([128, P + 1, KT], f32 if split3 else wdt)
            if split3:
                p16_sb = cpool.tile([128, P + 1, KT], f16)
                p16f_sb = cpool.tile([128, P + 1, KT], f32)
                plo_sb = cpool.tile([128, P + 1, KT], f16)
                p8_sb = cpool.tile([128, P + 1, KT], f8)
            vt_sb = cpool.tile([128, P, KT], f32)
            ph_sb = cpool.tile([128, KT], f32)
            t_sb = cpool.tile([128, 1], f32)
            pib_sb = cpool.tile([128, 1], f32)
            pib3_sb = cpool.tile([128, 1], f32)
            sin_sb = cpool.tile([128, KT], f32)
            s1_sb = cpool.tile([128, KT], f32)
            s2_sb = cpool.tile([128, KT], f32)
            # synapse rows split 16/4: the tail group keeps the last
            # synapses' work off the critical path, and separate tiles keep
            # every engine AP starting at partition 0
            SA = 16
            SB = S - SA
            s_a = cpool.tile([SA, JC], f32)
            s_b = cpool.tile([SB, JC], f32)
            v_sb = cpool.tile([P, JC], f32)
            pv_a = cpool.tile([SA, JC], f32)
            pv_b = cpool.tile([SB, JC], f32)
            er_a = cpool.tile([SA, 1], f32)
            er_b = cpool.tile([SB, 1], f32)
            ms_a = cpool.tile([SA, P], f32)
            ms_b = cpool.tile([SB, P], f32)
            snew_a = cpool.tile([SA, JC], f32)
            snew_b = cpool.tile([SB, JC], f32)
            sdec_a = cpool.tile([SA, JC], f32)
            sdec_b = cpool.tile([SB, JC], f32)
            wterm_a = cpool.tile([SA, JC], f32)
            wterm_b = cpool.tile([SB, JC], f32)
            isyn_a = cpool.tile([SA, JC], f32)
            isyn_b = cpool.tile([SB, JC], f32)
            gv_sb = cpool.tile([P, JC], f32)
            dv_sb = cpool.tile([P, JC], f32)
            vnew_sb = cpool.tile([P, JC], f32)
            rates4_sb = cpool.tile([P, JC], f32)
            err_sb = cpool.tile([P, 1], f32)

            # every small input DMA rides the SWDGE path (rate-prep ones
            # first) so the sync HWDGE FIFO is a pure back-to-back W stream
            nc.gpsimd.dma_start(out=t_sb, in_=t_in.to_broadcast((128, 1)))
            nc.gpsimd.dma_start(out=vt_sb, in_=vt_in)
            nc.gpsimd.dma_start(out=ph_sb, in_=ph_in)
            nc.gpsimd.dma_start(out=s_a, in_=s_in[0:SA, :])
            nc.gpsimd.dma_start(out=s_b, in_=s_in[SA:S, :])
            nc.gpsimd.dma_start(out=v_sb, in_=v_in)
            nc.gpsimd.dma_start(out=pv_a, in_=pv_in[0:SA, :])
            nc.gpsimd.dma_start(out=pv_b, in_=pv_in[SA:S, :])
            nc.gpsimd.dma_start(out=er_a, in_=er_in[0:SA, :])
            nc.gpsimd.dma_start(out=er_b, in_=er_in[SA:S, :])
            nc.gpsimd.dma_start(out=ms_a, in_=ms_in[0:SA, :])
            nc.gpsimd.dma_start(out=ms_b, in_=ms_in[SA:S, :])

            # ---- firing rates of all source populations -------------------
            # rows 0..P-1: sigmoid(v); row P: 0.5*(1+sin(2*pi*f*t + phase))
            nc.scalar.activation(
                out=rates_sb[:, 0:P, :], in_=vt_sb,
                func=mybir.ActivationFunctionType.Sigmoid,
            )
            # ScalarE Sin needs args in [-pi, pi]. a = phase + t_red lies in
            # [0, 4pi) (host pre-reduces the 2*pi*f*t scalar mod 2pi), so
            # subtract 2*pi*k with k = (sign(a-pi) + sign(a-3pi))/2 + 1.
            nc.vector.memset(pib_sb, -float(np.pi))
            nc.vector.memset(pib3_sb, -float(3.0 * np.pi))
            nc.vector.tensor_scalar(
                out=sin_sb, in0=ph_sb, scalar1=t_sb, scalar2=None,
                op0=mybir.AluOpType.add,
            )
            nc.scalar.activation(
                out=s1_sb, in_=sin_sb,
                func=mybir.ActivationFunctionType.Sign, bias=pib_sb,
            )
            nc.scalar.activation(
                out=s2_sb, in_=sin_sb,
                func=mybir.ActivationFunctionType.Sign, bias=pib3_sb,
            )
            nc.vector.tensor_add(s1_sb, s1_sb, s2_sb)
            nc.vector.tensor_scalar(
                out=s1_sb, in0=s1_sb,
                scalar1=-float(np.pi), scalar2=-float(2.0 * np.pi),
                op0=mybir.AluOpType.mult, op1=mybir.AluOpType.add,
            )
            nc.vector.tensor_add(sin_sb, sin_sb, s1_sb)
            nc.scalar.activation(
                out=sin_sb, in_=sin_sb,
                func=mybir.ActivationFunctionType.Sin,
            )
            nc.vector.tensor_scalar(
                out=rates_sb[:, P, :], in0=sin_sb,
                scalar1=0.5, scalar2=0.5,
                op0=mybir.AluOpType.mult, op1=mybir.AluOpType.add,
            )
            if split3:
                # p16 = fp16(rates); plo = (rates - p16) * 2^11 in fp16;
                # p8 = fp8(rates)
                nc.vector.tensor_copy(p16_sb, rates_sb)
                nc.vector.tensor_copy(p16f_sb, p16_sb)
                nc.vector.tensor_sub(p16f_sb, rates_sb, p16f_sb)
                nc.vector.tensor_scalar_mul(plo_sb, p16f_sb, SCALE_PLO)
                nc.vector.tensor_copy(p8_sb, rates_sb)

            # ---- per-synapse drive matmuls + synapse update ---------------
            for s in range(S):
                r = src_idx[s]
                if split3:
                    # one PSUM bank holds all three accumulators: the three
                    # streams run in PE column groups 0/1/2 (outputs at
                    # partitions 0/32/64) so their LDWEIGHTS/MATMUL pairs
                    # pipeline instead of serializing on one PE column
                    abc_ps = dpsum.tile([128, JC], f32, tag="dabc")
                    a_ps = abc_ps[0:1, :]
                    b_ps = abc_ps[32:33, :]
                    c_ps = abc_ps[64:65, :]
                    for c0 in range(0, KT, CH):
                        wh_tile = wpool.tile([128, CH, JC], f16, tag="wh")
                        wl_tile = wpool.tile([128, CH, JC], f8, tag="wl")
                        nc.sync.dma_start(out=wh_tile,
                                          in_=wh_in[:, s, c0:c0 + CH, :])
                        nc.sync.dma_start(out=wl_tile,
                                          in_=wl_in[:, s, c0:c0 + CH, :])
                        for k in range(CH):
                            kk = c0 + k
                            st, sp = kk == 0, kk == KT - 1
                            nc.tensor.matmul(
                                a_ps, p16_sb[:, r, kk:kk + 1],
                                wh_tile[:, k, :], start=st, stop=sp,
                                tile_position=(0, 0))
                            nc.tensor.matmul(
                                b_ps, plo_sb[:, r, kk:kk + 1],
                                wh_tile[:, k, :], start=st, stop=sp,
                                tile_position=(0, 32))
                            nc.tensor.matmul(
                                c_ps, p8_sb[:, r, kk:kk + 1],
                                wl_tile[:, k, :], start=st, stop=sp,
                                tile_position=(0, 64))
                else:
                    drive_ps = dpsum.tile([1, JC], f32, tag="drive")
                    for c0 in range(0, KT, CH):
                        w_tile = wpool.tile([128, CH, JC], wdt, tag="w")
                        nc.sync.dma_start(out=w_tile,
                                          in_=w_in[:, s, c0:c0 + CH, :])
                        for k in range(CH):
                            kk = c0 + k
                            nc.tensor.matmul(
                                drive_ps, rates_sb[:, r, kk:kk + 1],
                                w_tile[:, k, :],
                                start=(kk == 0), stop=(kk == KT - 1),
                            )
                # s_new partial: DT * drive. ACT stages PSUM -> SBUF at
                # partition 0 (engines can't start mid-partition), then a
                # tiny SBUF->SBUF DMA scatters it to row s.
                stage_sb = stage_pool.tile([1, JC], f32, tag="stage")
                if split3:
                    # drive = a + b/2^11 + c/2^16  (scales fold with DT)
                    tb_sb = stage_pool.tile([1, JC], f32, tag="tb")
                    tc_sb = stage_pool.tile([1, JC], f32, tag="tc")
                    nc.scalar.mul(stage_sb, a_ps, DT)
                    nc.vector.tensor_scalar_mul(tb_sb, b_ps, DT / SCALE_PLO)
                    nc.vector.tensor_scalar_mul(tc_sb, c_ps, DT / SCALE_LO)
                    nc.vector.tensor_add(stage_sb, stage_sb, tb_sb)
                    nc.vector.tensor_add(stage_sb, stage_sb, tc_sb)
                else:
                    nc.scalar.mul(stage_sb, drive_ps, DT)
                # SWDGE path: keeps this dependent little DMA out of the
                # HWDGE FIFOs that stream the W chunks
                if s < SA:
                    nc.gpsimd.dma_start(out=snew_a[s:s + 1, :], in_=stage_sb)
                else:
                    nc.gpsimd.dma_start(out=snew_b[s - SA:s - SA + 1, :],
                                        in_=stage_sb)

            # ---- synapse state + currents (two row groups; group a runs
            # while the last synapses are still streaming) -------------------
            nc.vector.tensor_scalar_mul(sdec_a, s_a, DECAY)
            nc.vector.tensor_scalar_mul(sdec_b, s_b, DECAY)
            # wterm = E_rev - post_v
            nc.vector.tensor_scalar(
                out=wterm_a, in0=pv_a, scalar1=-1.0, scalar2=er_a,
                op0=mybir.AluOpType.mult, op1=mybir.AluOpType.add,
            )
            nc.vector.tensor_scalar(
                out=wterm_b, in0=pv_b, scalar1=-1.0, scalar2=er_b,
                op0=mybir.AluOpType.mult, op1=mybir.AluOpType.add,
            )
            itot_ps = apsum.tile([P, JC], f32, tag="acc")
            gtot_ps = apsum.tile([P, JC], f32, tag="acc")

            nc.vector.tensor_add(snew_a, snew_a, sdec_a)
            nc.sync.dma_start(out=snew_out[0:SA, :], in_=snew_a)
            nc.vector.tensor_mul(isyn_a, snew_a, wterm_a)
            nc.tensor.matmul(itot_ps, ms_a, isyn_a, start=True, stop=False)
            nc.tensor.matmul(gtot_ps, ms_a, snew_a, start=True, stop=False)

            nc.vector.tensor_add(snew_b, snew_b, sdec_b)
            nc.sync.dma_start(out=snew_out[SA:S, :], in_=snew_b)
            nc.vector.tensor_mul(isyn_b, snew_b, wterm_b)
            nc.tensor.matmul(itot_ps, ms_b, isyn_b, start=False, stop=True)
            nc.tensor.matmul(gtot_ps, ms_b, snew_b, start=False, stop=True)

            # ---- population integration -----------------------------------
            nc.vector.tensor_mul(gv_sb, gtot_ps, v_sb)
            nc.vector.tensor_sub(dv_sb, itot_ps, gv_sb)
            nc.vector.tensor_sub(dv_sb, dv_sb, v_sb)
            nc.vector.tensor_scalar_mul(dv_sb, dv_sb, 1.0 / TAU_M)
            nc.vector.tensor_reduce(
                out=err_sb, in_=dv_sb, axis=mybir.AxisListType.X,
                op=mybir.AluOpType.add, apply_absolute_value=True,
            )
            nc.vector.tensor_scalar_mul(vnew_sb, dv_sb, DT)
            nc.vector.tensor_add(vnew_sb, vnew_sb, v_sb)
            nc.scalar.activation(
                out=rates4_sb, in_=vnew_sb,
                func=mybir.ActivationFunctionType.Sigmoid,
            )

            nc.sync.dma_start(out=vnew_out, in_=vnew_sb)
            nc.sync.dma_start(out=rates_out, in_=rates4_sb)
            nc.sync.dma_start(out=errp_out, in_=err_sb)

    nc.compile()
    return nc


def _get_program(src_idx, tgt_idx, mode):
    key = (tuple(int(x) for x in src_idx), tuple(int(x) for x in tgt_idx), mode)
    if key not in _PROGRAM_CACHE:
        _PROGRAM_CACHE[key] = _build_program(key[0], key[1], mode)
    return _PROGRAM_CACHE[key]


def _prep_inputs(t, v, s, W, E_rev, phase, src_idx, tgt_idx, mode=None):
    """Host-side shard/layout prep (gathers/transposes/precision encode)."""
    import ml_dtypes

    mode = mode or W_MODE
    f32 = np.float32
    v2 = np.asarray(v, f32)[:, 0, :]                      # [P, N]
    s2 = np.asarray(s, f32)[:, 0, :]                      # [S, N]
    W = np.asarray(W, f32)
    tgt = np.asarray(tgt_idx)

    # lhsT layouts: element [p, ..., k] = x[..., k*128 + p]
    vt = np.ascontiguousarray(v2.reshape(P, KT, 128).transpose(2, 0, 1))
    ph = np.ascontiguousarray(
        np.asarray(phase, f32).reshape(KT, 128).transpose(1, 0))
    msel = np.zeros((S, P), f32)
    msel[np.arange(S), tgt] = 1.0
    erev = np.asarray(E_rev, f32).reshape(S, 1)
    # t enters as the fp32 angle 2*pi*f*t, pre-reduced mod 2pi (one host
    # scalar op) so the on-device sin range reduction only spans [0, 4pi).
    tr = f32(2.0 * np.pi * IN_FREQ) * np.asarray(t, f32).reshape(-1)[0]
    if tr >= f32(2.0 * np.pi) or tr < 0.0:
        tr = f32(np.float64(tr) % (2.0 * np.pi))
    t_arr = np.asarray(tr, f32).reshape(1, 1)

    # W[s, i, j] with i = kt*128 + p, j = c*JC + jj  ->  per-core [128, S, KT, JC]
    if mode == "split3":
        WH = W.astype(np.float16)
        WL = ((W - WH.astype(f32)) * f32(SCALE_LO)).astype(ml_dtypes.float8_e4m3)
        WH5 = WH.reshape(S, KT, 128, N_CORES, JC)
        WL5 = WL.reshape(S, KT, 128, N_CORES, JC)
    else:
        W5 = W.reshape(S, KT, 128, N_CORES, JC)

    in_maps = []
    for c in range(N_CORES):
        sl = slice(c * JC, (c + 1) * JC)
        vc = np.ascontiguousarray(v2[:, sl])
        m = dict(
            vt=vt, ph=ph, t=t_arr,
            s=np.ascontiguousarray(s2[:, sl]),
            v=vc,
            pv=np.ascontiguousarray(vc[tgt]),
            er=erev, ms=msel,
        )
        if mode == "split3":
            m["wh"] = np.ascontiguousarray(WH5[:, :, :, c, :].transpose(2, 0, 1, 3))
            m["wl"] = np.ascontiguousarray(WL5[:, :, :, c, :].transpose(2, 0, 1, 3))
        else:
            m["w"] = np.ascontiguousarray(W5[:, :, :, c, :].transpose(2, 0, 1, 3))
        in_maps.append(m)
    return in_maps


def kernel(t, v, s, W, E_rev, phase, src_idx, tgt_idx):
    global LAST_EXEC_NS, LAST_RESULTS
    nc = _get_program(src_idx, tgt_idx, W_MODE)
    in_maps = _prep_inputs(t, v, s, W, E_rev, phase, src_idx, tgt_idx)

    kwargs = {}
    if TRACE:
        _install_ntff_hook()
        kwargs = dict(trace=True, trace_cores=list(range(N_CORES)))
    res = run_bass_kernel_spmd(nc, in_maps, core_ids=list(range(N_CORES)),
                               **kwargs)
    LAST_EXEC_NS = res.exec_time_ns
    LAST_RESULTS = res

    # ---- unshard -----------------------------------------------------------
    f32 = np.float32
    s_new = np.empty((S, B, N), f32)
    v_new = np.empty((P, B, N), f32)
    rates = np.empty((P, N), f32)
    err_acc = 0.0
    for c in range(N_CORES):
        r = res.results[c]
        sl = slice(c * JC, (c + 1) * JC)
        s_new[:, 0, sl] = r["snew"]
        v_new[:, 0, sl] = r["vnew"]
        rates[:, sl] = r["rates"]
        err_acc += float(r["errp"].sum())
    err = np.asarray(err_acc / N, f32)
    out = np.concatenate(
        [rates.reshape(1, P * N), err.reshape(1, 1)], axis=1).astype(f32)
    return out, v_new, s_new
